# revision 64
# baseline (speedup 1.0000x reference)
"""Multi-head attention (B=2, S=2048, H=1024, 16 heads x 64d) on 8 trn2 cores.

Sharding: tensor-parallel over heads (2 heads/core). Each core computes the
qkv projection for its 384 output features, attention for its 2 heads, and a
partial o_proj ([4096,1024] over its 128-feature slice). Host sums the 8
partials (fp16) and adds b_o.

v2 (fused heads): the two heads are processed in lockstep per (batch, query
chunk) unit. Scores for h0/h1 are emitted back-to-back as 64x128 row-tiled
matmuls at tile positions (0,0)/(64,0) into separate PSUM banks so the PE
array halves can overlap; one [128,1024] exp covers both heads per k-slab.
o_proj output and final host sum are fp16. Emission software-pipelines S/PV
around the exp and drains a fine-grained filler queue (qkv chunks /
V transposes / o_proj) inside exp shadows.

v3 (detached normalization, 306us -> 234us): the per-unit softmax
normalization is decoupled from the PSUM accumulators and from the exp
stream. At the next unit's kk=0 a DVE-only stash copies unnormalized OT and
the two den rows to SBUF (releasing the o_ps banks ~2.6us in, in parallel
with the exps); Ln / rec-exp+broadcast / in-place fp16 muls then run as
thunks at kk=2/4 with no PSUM coupling. The baseline emitted 2xLn + rec-exp
between the last PV and the next unit's scores, which stalled the PE ~2.1us
per unit and HAM-rethrottled the clock to 1.2GHz for ~3-7us afterwards
(64us of throttled PE time -> ~19us). Other changes: [128,128] merged
both-heads V transposes (32 instead of 64), fp16 e01/rec2 broadcast matmul,
region-split first DMAs so the first qkv matmul starts ~4us earlier, only
chunk 0 inline before unit 0 (first exp at ~25us instead of ~39us), each
unit's oproj group enqueued from its muls thunk (emission-order safe),
per-token-tile flush DMAs, and a short-latency inline tail for the final
unit (PSUM-direct norm + DVE/ACT-alternating oproj casts).
"""
import sys

sys.path.insert(0, "/opt/trn_rl_repo")
import numpy as np

NHEADS = 16
HEAD_DIM = 64
HIDDEN = 1024
QKV = NHEADS * HEAD_DIM  # 1024
SCALING = HEAD_DIM ** -0.5
B = 2
S = 2048
T = B * S  # 4096
NCORES = 8
HPC = NHEADS // NCORES  # 2 heads per core
FEAT = HPC * HEAD_DIM  # 128
CHUNK = 512
NCHUNK = S // CHUNK  # 4 per batch
KSLABS = HIDDEN // 128  # 8
SSLABS = S // 128  # 16
D1 = HEAD_DIM + 1  # 65

# toggles for A/B experiments
TILED_SCORES = True   # fused-head 64x128 row-tiled score pairs
MERGED_VTRANS = True  # [128,128] both-heads V transposes (32 vs 64)

_CACHE = {}
LAST_RESULT = None  # BassKernelResults of the most recent kernel() call


def _split_waits(nc, keep=1):
    """Hoist excess per-instruction sem waits into standalone EventSemaphore
    instructions (walrus codegen has small per-opcode wait budgets)."""
    import bass_rust
    import concourse.mybir as mybir

    n_new = 0
    for f in nc.m.functions:
        for blk in f.blocks:
            out = []
            changed = False
            for inst in blk.instructions:
                si = inst.sync_info
                waits = list(si.on_wait) if si is not None else []
                if len(waits) > keep:
                    excess = waits[: len(waits) - keep]
                    kept = waits[len(waits) - keep:]
                    for w in excess:
                        out.append(mybir.InstEventSemaphore(
                            name=f"{inst.name}-esw{n_new}",
                            engine=inst.engine,
                            sync_info=bass_rust.SyncInfo(on_wait=[w], on_update=[]),
                        ))
                        n_new += 1
                    inst.sync_info = bass_rust.SyncInfo(
                        on_wait=kept, on_update=list(si.on_update))
                    changed = True
                out.append(inst)
            if changed:
                blk.instructions = out
    return n_new


def _build(reps=1):
    import concourse.bass as bass
    import concourse.mybir as mybir
    import concourse.tile as tile

    from concourse.masks import make_identity
    f32 = mybir.dt.float32
    f32r = mybir.dt.float32r
    f16 = mybir.dt.float16
    Exp = mybir.ActivationFunctionType.Exp
    Ln = mybir.ActivationFunctionType.Ln

    nc = bass.Bass()
    xT = nc.dram_tensor("xT", [HIDDEN, T], f16, kind="ExternalInput")
    wqkvT = nc.dram_tensor("wqkvT", [HIDDEN, 3 * FEAT], f16, kind="ExternalInput")
    bqkv = nc.dram_tensor("bqkv", [FEAT, 3], f32, kind="ExternalInput")
    woT = nc.dram_tensor("woT", [FEAT, HIDDEN], f16, kind="ExternalInput")
    out_d = nc.dram_tensor("out", [T, HIDDEN], f16, kind="ExternalOutput")

    with tile.TileContext(nc) as tc, nc.allow_low_precision(reason="fp16 matmuls"):
        with (
            tc.tile_pool(name="sing", bufs=1) as sing,
            tc.tile_pool(name="xp", bufs=4) as xp,
            tc.tile_pool(name="pp", bufs=3) as pp,
            tc.tile_pool(name="stg", bufs=4) as stg,
            tc.tile_pool(name="sm", bufs=2) as sm,
            tc.tile_pool(name="dnp", bufs=2) as dnp,
            tc.tile_pool(name="op", bufs=2) as op,
            tc.tile_pool(name="ps_mm", bufs=2, space="PSUM") as ps_mm,
            tc.tile_pool(name="ps_s", bufs=2, space="PSUM") as ps_s,
            tc.tile_pool(name="ps_o", bufs=2, space="PSUM") as ps_o,
        ):
            wq_sb = sing.tile([128, KSLABS, 3 * FEAT], f16, tag="wq")
            wo_sb = sing.tile([FEAT, HIDDEN], f16, tag="wo")
            bq_sb = sing.tile([FEAT, 3], f32, tag="bq")
            QT = sing.tile([128, T], f16, tag="qt")
            KT = sing.tile([128, T], f16, tag="kt")
            VT = sing.tile([128, T], f32, tag="vt")
            OT = sing.tile([128, T], f16, tag="ot")
            Vaug = sing.tile([128, B, HPC, SSLABS, D1], f16, tag="va")

            def qkv_dma_early(n, split=1):
                """Chunk DMA, optionally split by k-slab halves so the first
                qkv matmuls can start before the whole chunk lands."""
                xc = xp.tile([128, KSLABS, CHUNK], f16, tag="xc", name="xc")
                xr = xT[:].rearrange("(s p) t -> p s t", p=128)
                sl = KSLABS // split
                for i in range(split):
                    nc.gpsimd.dma_start(
                        out=xc[:, i * sl:(i + 1) * sl, :],
                        in_=xr[:, i * sl:(i + 1) * sl,
                               n * CHUNK:(n + 1) * CHUNK])
                return xc

            wq_r = wqkvT[:].rearrange("(s p) f -> p s f", p=128)
            # finest splits first: MM(slab s) unblocks as soon as its own
            # wq+xc regions land (region-granular tile deps)
            nc.sync.dma_start(out=wq_sb[:, 0:2, 0:FEAT],
                              in_=wq_r[:, 0:2, 0:FEAT])
            xc01 = [qkv_dma_early(0, split=4), qkv_dma_early(1)]
            nc.sync.dma_start(out=wq_sb[:, 2:4, 0:FEAT],
                              in_=wq_r[:, 2:4, 0:FEAT])
            nc.sync.dma_start(out=wq_sb[:, 4:KSLABS, 0:FEAT],
                              in_=wq_r[:, 4:KSLABS, 0:FEAT])
            for m3 in range(1, 3):
                nc.sync.dma_start(out=wq_sb[:, :, m3 * FEAT:(m3 + 1) * FEAT],
                                  in_=wq_r[:, :, m3 * FEAT:(m3 + 1) * FEAT])
            nc.sync.dma_start(out=bq_sb, in_=bqkv[:])
            nc.sync.dma_start(out=wo_sb, in_=woT[:])
            ident = sing.tile([128, 128], f32, tag="id")
            make_identity(nc, ident)
            # e64a: ones at row 64 — e64a.T @ dn broadcasts the den row to
            # 64 output partitions; used col-tiled per head
            e64a = sing.tile([D1, HEAD_DIM], f16, tag="e64")
            dn2 = sing.tile([D1, 2, 2 * CHUNK], f16, tag="dn2")
            nc.vector.memset(e64a, 0.0)
            nc.vector.memset(e64a[HEAD_DIM:D1, :], 1.0)
            nc.vector.memset(dn2, 0.0)
            vst = stg.tile([128, B * HPC * SSLABS], f32, tag="vst")
            nc.vector.memset(vst, 1.0)
            nc.vector.tensor_copy(Vaug[:, :, :, :, HEAD_DIM:D1], vst)

            xT_c = xT[:].rearrange("(s p) t -> p s t", p=128)

            from collections import deque
            filler = deque()
            fillmm = deque()  # items guaranteed to start with a PE matmul

            def drain(n=1):
                for _ in range(n):
                    if filler:
                        filler.popleft()()
                    elif fillmm:
                        fillmm.popleft()()

            def drain_mm(n=1):
                for _ in range(n):
                    if fillmm:
                        fillmm.popleft()()
                    elif filler:
                        filler.popleft()()

            # emission-order guards: count-based drain pacing alone cannot
            # guarantee a consumer is EMITTED after its producer thunk (tile
            # deps only see already-emitted instructions)
            ready = set()

            def mark(key):
                ready.add(key)

            def need(key):
                while key not in ready:
                    assert filler or fillmm, f"need({key}): queues empty"
                    drain(1)

            def qkv_dma(b, n):
                g = b * NCHUNK + n
                xc = xp.tile([128, KSLABS, CHUNK], f16, tag="xc", name="xc")
                for i in range(2):
                    nc.gpsimd.dma_start(
                        out=xc[:, 4 * i:4 * i + 4, :],
                        in_=xT_c[:, 4 * i:4 * i + 4,
                                 g * CHUNK:(g + 1) * CHUNK])
                return xc

            def qkv_feat_half(b, n, m, xc, half, box):
                g = b * NCHUNK + n
                lo, hi = g * CHUNK, (g + 1) * CHUNK
                dest = (QT, KT, VT)[m]
                if half == 0:
                    box["acc"] = ps_mm.tile([128, CHUNK], f32, tag="mm",
                                            name="acc")
                acc = box["acc"]
                s0 = half * (KSLABS // 2)
                for s in range(s0, s0 + KSLABS // 2):
                    nc.tensor.matmul(
                        acc, wq_sb[:, s, m * FEAT:(m + 1) * FEAT], xc[:, s, :],
                        start=(s == 0), stop=(s == KSLABS - 1))
                if half == 1:
                    nc.vector.tensor_scalar_add(
                        dest[:, lo:hi], acc, bq_sb[:, m:m + 1])

            def qkv_feat(b, n, m, xc):
                box = {}
                qkv_feat_half(b, n, m, xc, 0, box)
                qkv_feat_half(b, n, m, xc, 1, box)

            def vtrans_k(b, k):
                """[128,128] PE transpose covering both heads at once."""
                tp = ps_mm.tile([128, CHUNK], f32, tag="mm", name="tp")
                nc.tensor.transpose(
                    tp[:, 0:128],
                    VT[:, b * S + 128 * k: b * S + 128 * (k + 1)],
                    ident)
                nc.vector.tensor_copy(
                    Vaug[:, b, :, k, 0:HEAD_DIM],
                    tp[:, 0:128].rearrange("p (h d) -> p h d", h=HPC))

            def norm_thunks(b, qc, o_ps, oproj_group=None):
                """Detached normalization, 4 phases dropped at kk=0/2/4/6 of
                the NEXT unit. Phase 0 (all DVE) stashes unnormalized OT and
                the two den rows to SBUF, releasing the o_ps PSUM banks
                ~2.6us into the next unit without touching ACT. Ln / rec /
                in-place fp16 muls then run with no PSUM coupling, so the
                softmax exps stream uninterrupted and the PE never idles
                long enough to re-throttle. The unit's oproj group is
                enqueued from inside the muls thunk (emission-order-safe)."""
                qlo = b * S + qc * CHUNK
                qsl = slice(qlo, qlo + CHUNK)
                par = qc % 2
                box = {}

                def stash():
                    for h in range(HPC):
                        nc.vector.tensor_copy(
                            OT[64 * h:64 * h + 64, qsl],
                            o_ps[h][0:HEAD_DIM, :])
                        nc.vector.tensor_copy(
                            dn2[HEAD_DIM:D1, par,
                                h * CHUNK:(h + 1) * CHUNK],
                            o_ps[h][HEAD_DIM:D1, :])

                def t_ln():
                    # broadcast-first: den rows fan out to all 128
                    # partitions via two col-tiled ones-row matmuls, then
                    # Ln and exp run as single full-width [128,512] ACT ops
                    b_ps = ps_mm.tile([128, CHUNK], f32, tag="mm",
                                      name="bps")
                    for h in range(HPC):
                        nc.tensor.matmul(
                            b_ps[64 * h:64 * h + 64, :], e64a,
                            dn2[:, par, h * CHUNK:(h + 1) * CHUNK],
                            start=True, stop=True)
                    lnt = sm.tile([128, CHUNK], f32, tag="ln", name="lnt")
                    nc.scalar.activation(out=lnt, in_=b_ps, func=Ln)
                    box["lnt"] = lnt

                def t_rec():
                    rb = sm.tile([128, CHUNK], f16, tag="rb", name="rb")
                    nc.scalar.activation(out=rb, in_=box["lnt"], func=Exp,
                                         scale=-1.0)
                    box["rb"] = rb

                def t_mul():
                    for h in range(HPC):
                        nc.vector.tensor_mul(
                            OT[64 * h:64 * h + 64, qsl],
                            OT[64 * h:64 * h + 64, qsl],
                            box["rb"][64 * h:64 * h + 64, :])
                    if oproj_group is not None:
                        fillmm.extend(oproj_group_thunks(oproj_group))

                def t_rec_mul():
                    t_rec()
                    t_mul()

                return [stash, t_ln, t_rec_mul]

            def attn_unit(b, qc, extra_drain=1, finish_prev=None,
                          last=False):
                """Fused-head unit: 16 k-slab steps, software-pipelined."""
                qlo = b * S + qc * CHUNK
                qsl = slice(qlo, qlo + CHUNK)
                o_ps = [ps_o.tile([D1, CHUNK], f32, tag="o",
                                  name=f"o{h}_{b}_{qc}") for h in range(HPC)]

                def s_step(k):
                    s_ps = ps_s.tile([128, HPC, CHUNK], f32, tag="s",
                                     name="s_ps")
                    for h in range(HPC):
                        nc.tensor.matmul(
                            s_ps[:, h, :],
                            KT[64 * h:64 * h + 64,
                               b * S + 128 * k: b * S + 128 * (k + 1)],
                            QT[64 * h:64 * h + 64, qsl],
                            start=True, stop=True)
                    ptk = pp.tile([128, HPC, CHUNK], f16, tag="pt", name="pt")
                    nc.scalar.activation(out=ptk, in_=s_ps, func=Exp)
                    return ptk

                def pv_step(k, ptk):
                    for h in range(HPC):
                        nc.tensor.matmul(
                            o_ps[h], Vaug[:, b, h, k, :], ptk[:, h, :],
                            start=(k == 0), stop=(k == SSLABS - 1))

                # 2-slab groups: [sc(k), sc(k+1)] then [pv(k), pv(k+1)],
                # pipelined one group deep — keeps TensorE in each tile mode
                # for 4 matmuls at a time (mode switches are ~100-250ns).
                need(("qt", b, qc))
                prev = None
                for kk in range(0, SSLABS, 2):
                    need(("kt", b, (kk + 1) // 4))
                    cur = (s_step(kk), s_step(kk + 1))
                    if finish_prev is not None and kk < 6:
                        finish_prev[kk // 2]()
                        if kk == 0:
                            drain_mm(2)
                    drain(extra_drain)
                    if prev is not None:
                        need(("va", b, (kk - 1) // 4))
                        pv_step(kk - 2, prev[0])
                        pv_step(kk - 1, prev[1])
                        drain(extra_drain)
                    prev = cur
                need(("va", b, NCHUNK - 1))
                pv_step(SSLABS - 2, prev[0])
                drain(extra_drain)
                if not last:
                    pv_step(SSLABS - 1, prev[1])
                    return norm_thunks(b, qc, o_ps,
                                       oproj_group=b * NCHUNK + qc)

                # Final unit: shortest-latency tail. Den copies interleaved
                # between the last PVs, broadcast-first norm, then the oproj
                # group inline with DVE/ACT-alternating casts.
                k15 = SSLABS - 1
                par = qc % 2
                nc.tensor.matmul(o_ps[0], Vaug[:, b, 0, k15, :],
                                 prev[1][:, 0, :], start=False, stop=True)
                nc.vector.tensor_copy(dn2[HEAD_DIM:D1, par, 0:CHUNK],
                                      o_ps[0][HEAD_DIM:D1, :])
                nc.tensor.matmul(o_ps[1], Vaug[:, b, 1, k15, :],
                                 prev[1][:, 1, :], start=False, stop=True)
                nc.vector.tensor_copy(dn2[HEAD_DIM:D1, par, CHUNK:2 * CHUNK],
                                      o_ps[1][HEAD_DIM:D1, :])
                b_ps = ps_mm.tile([128, CHUNK], f32, tag="mm", name="bps")
                for h in range(HPC):
                    nc.tensor.matmul(
                        b_ps[64 * h:64 * h + 64, :], e64a,
                        dn2[:, par, h * CHUNK:(h + 1) * CHUNK],
                        start=True, stop=True)
                lnt = sm.tile([128, CHUNK], f32, tag="ln", name="lntL")
                nc.scalar.activation(out=lnt, in_=b_ps, func=Ln)
                rb = sm.tile([128, CHUNK], f32, tag="rb", name="rb")
                nc.scalar.activation(out=rb, in_=lnt, func=Exp, scale=-1.0)
                # quarter-split: normalize 128 tokens, project and flush them
                # while the next quarter's muls run — shortens the tail chain
                g = b * NCHUNK + qc
                ost = op.tile([128, 4, HIDDEN], f16, tag="ost", name="ostL")
                for jj in range(4):
                    cs = slice(128 * jj, 128 * (jj + 1))
                    qq = slice(qlo + 128 * jj, qlo + 128 * (jj + 1))
                    for h in range(HPC):
                        nc.vector.tensor_mul(
                            OT[64 * h:64 * h + 64, qq],
                            o_ps[h][0:HEAD_DIM, cs],
                            rb[64 * h:64 * h + 64, cs])
                    for nh in range(HIDDEN // CHUNK):
                        oproj_half(g, jj, nh, ost,
                                   cast_scalar=(nh == 1))
                    nc.sync.dma_start(
                        out=out_d[512 * g + 128 * jj:
                                  512 * g + 128 * (jj + 1), :],
                        in_=ost[:, jj, :])
                return None

            def oproj_half(j, jj, nh, ost, cast_scalar=False):
                t = 4 * j + jj
                acc = ps_mm.tile([128, CHUNK], f32, tag="mm", name="acc2")
                nc.tensor.matmul(
                    acc, OT[:, 128 * t:128 * (t + 1)],
                    wo_sb[:, nh * CHUNK:(nh + 1) * CHUNK],
                    start=True, stop=True)
                if cast_scalar:
                    nc.scalar.copy(
                        ost[:, jj, nh * CHUNK:(nh + 1) * CHUNK], acc)
                else:
                    nc.vector.tensor_copy(
                        ost[:, jj, nh * CHUNK:(nh + 1) * CHUNK], acc)

            def oproj_group_thunks(j):
                # token tiles 4j..4j+3 (tokens 512j..512j+512); per-jj
                # flush DMAs so the tail cast/DMA pipeline stays overlapped
                box = {}

                def first():
                    box["ost"] = op.tile([128, 4, HIDDEN], f16, tag="ost",
                                         name="ost")
                    oproj_half(j, 0, 0, box["ost"])

                def flush1(jj):
                    nc.sync.dma_start(
                        out=out_d[512 * j + 128 * jj:512 * j + 128 * (jj + 1),
                                  :],
                        in_=box["ost"][:, jj, :])

                thunks = []
                for jj in range(4):
                    for nh in range(HIDDEN // CHUNK):
                        if jj == 0 and nh == 0:
                            thunks.append(first)
                        else:
                            thunks.append(
                                lambda jj=jj, nh=nh:
                                oproj_half(j, jj, nh, box["ost"]))
                    thunks.append(lambda jj=jj: flush1(jj))
                return thunks

            def queue_chunk(b, n, xc=None):
                """Queue one qkv chunk (dma + 3 projections + V transposes)."""
                box = {}

                if xc is not None:
                    box["xc"] = xc
                else:
                    def dma_thunk():
                        box["xc"] = qkv_dma(b, n)

                    filler.append(dma_thunk)
                for m in range(3):
                    fbox = {}

                    def feat_thunk(m=m, fbox=fbox, half=None):
                        qkv_feat_half(b, n, m, box["xc"], half, fbox)
                        if half == 1:
                            mark((("qt", "kt", "vt")[m], b, n))

                    for half in range(2):
                        filler.append(
                            lambda half=half, ft=feat_thunk: ft(half=half))

                def vt_thunk(k):
                    vtrans_k(b, k)
                    if k == 4 * n + 3:
                        mark(("va", b, n))

                for k in range(4 * n, 4 * n + 4):
                    filler.append(lambda k=k: vt_thunk(k))

            # ---- emission: qkv(b0 ch0-1) up front, then fused-head units
            # with fine-grained filler (qkv / vtrans / norm / oproj) drained
            # inside exp shadows ----
            for _rep in range(reps):
                # inline only chunk-0 q/k feats — the minimum for unit 0's
                # first score pair; v-feats + vtranses drain as the first
                # fillers inside unit 0 (readiness guards enforce emission
                # order), so the exp pipeline starts ~10us earlier
                xc0 = xc01[0] if _rep == 0 else qkv_dma(0, 0)
                for m in range(2):
                    qkv_feat(0, 0, m, xc0)
                    mark((("qt", "kt")[m], 0, 0))
                vbox = {}

                def v0_thunk(half):
                    qkv_feat_half(0, 0, 2, xc0, half, vbox)
                    if half == 1:
                        mark(("vt", 0, 0))

                for half in range(2):
                    filler.append(lambda half=half: v0_thunk(half))

                def vt0_thunk(k):
                    vtrans_k(0, k)
                    if k == 3:
                        mark(("va", 0, 0))

                for k in range(4):
                    filler.append(lambda k=k: vt0_thunk(k))

                queue_chunk(0, 1, xc=xc01[1] if _rep == 0 else None)
                for n in range(2, NCHUNK):
                    queue_chunk(0, n)
                for n in range(NCHUNK):
                    queue_chunk(1, n)

                fin = None
                for qc in range(NCHUNK):  # batch 0
                    fin = attn_unit(0, qc,
                                    extra_drain=2 if qc < 2 else 1,
                                    finish_prev=fin)
                for qc in range(NCHUNK):  # batch 1
                    fin = attn_unit(1, qc, finish_prev=fin,
                                    extra_drain=2 if qc == NCHUNK - 1 else 1,
                                    last=(qc == NCHUNK - 1))
                assert fin is None
                while filler or fillmm:
                    drain(1)

    _split_waits(nc)
    return nc


def _make_in_maps(hidden_states, w_qkv, b_qkv, w_o, b_o):
    x16 = np.ascontiguousarray(
        np.asarray(hidden_states, dtype=np.float32).reshape(T, HIDDEN).T
    ).astype(np.float16)
    w_qkv = np.asarray(w_qkv, dtype=np.float32)
    b_qkv = np.asarray(b_qkv, dtype=np.float32)
    w_o = np.asarray(w_o, dtype=np.float32)

    in_maps = []
    for c in range(NCORES):
        rq = slice(c * FEAT, (c + 1) * FEAT)
        wq = w_qkv[0:QKV][rq] * SCALING
        wk = w_qkv[QKV:2 * QKV][rq]
        wv = w_qkv[2 * QKV:3 * QKV][rq]
        bq = b_qkv[0:QKV][rq] * SCALING
        bk = b_qkv[QKV:2 * QKV][rq]
        bv = b_qkv[2 * QKV:3 * QKV][rq]
        in_maps.append({
            "xT": x16,
            "wqkvT": np.ascontiguousarray(
                np.concatenate([wq, wk, wv], axis=0).T).astype(np.float16),
            "bqkv": np.ascontiguousarray(np.stack([bq, bk, bv], axis=1)),
            "woT": np.ascontiguousarray(w_o[:, rq].T).astype(np.float16),
        })
    return in_maps


def kernel(hidden_states, w_qkv, b_qkv, w_o, b_o):
    global LAST_RESULT
    from concourse.bass_utils import run_bass_kernel_spmd
    import os

    if "nc" not in _CACHE:
        _CACHE["nc"] = _build()
    nc = _CACHE["nc"]

    b_o = np.asarray(b_o, dtype=np.float32)
    in_maps = _make_in_maps(hidden_states, w_qkv, b_qkv, w_o, b_o)

    trace = bool(os.environ.get("KERNEL_TRACE"))
    res = run_bass_kernel_spmd(nc, in_maps, list(range(NCORES)), trace=trace)
    LAST_RESULT = res

    acc = np.zeros((T, HIDDEN), dtype=np.float32)
    for c in range(NCORES):
        acc += res.results[c]["out"]
    out = (acc + b_o).astype(np.float32).reshape(B, S, HIDDEN)
    return out



# revision 65
# speedup vs baseline: 1.0165x; 1.0165x over previous
"""Multi-head attention (B=2, S=2048, H=1024, 16 heads x 64d) on 8 trn2 cores.

Sharding: tensor-parallel over heads (2 heads/core). Each core computes the
qkv projection for its 384 output features, attention for its 2 heads, and a
partial o_proj ([4096,1024] over its 128-feature slice). Host sums the 8
partials (fp16) and adds b_o.

v2 (fused heads): the two heads are processed in lockstep per (batch, query
chunk) unit. Scores for h0/h1 are emitted back-to-back as 64x128 row-tiled
matmuls at tile positions (0,0)/(64,0) into separate PSUM banks so the PE
array halves can overlap; one [128,1024] exp covers both heads per k-slab.
o_proj output and final host sum are fp16. Emission software-pipelines S/PV
around the exp and drains a fine-grained filler queue (qkv chunks /
V transposes / o_proj) inside exp shadows.

v3 (detached normalization, 306us -> 234us): the per-unit softmax
normalization is decoupled from the PSUM accumulators and from the exp
stream. At the next unit's kk=0 a DVE-only stash copies unnormalized OT and
the two den rows to SBUF (releasing the o_ps banks ~2.6us in, in parallel
with the exps); Ln / rec-exp+broadcast / in-place fp16 muls then run as
thunks at kk=2/4 with no PSUM coupling. The baseline emitted 2xLn + rec-exp
between the last PV and the next unit's scores, which stalled the PE ~2.1us
per unit and HAM-rethrottled the clock to 1.2GHz for ~3-7us afterwards
(64us of throttled PE time -> ~19us). Other changes: [128,128] merged
both-heads V transposes (32 instead of 64), fp16 e01/rec2 broadcast matmul,
region-split first DMAs so the first qkv matmul starts ~4us earlier, only
chunk 0 inline before unit 0 (first exp at ~25us instead of ~39us), each
unit's oproj group enqueued from its muls thunk (emission-order safe),
per-token-tile flush DMAs, and a short-latency inline tail for the final
unit (PSUM-direct norm + DVE/ACT-alternating oproj casts).
"""
import sys

sys.path.insert(0, "/opt/trn_rl_repo")
import numpy as np

NHEADS = 16
HEAD_DIM = 64
HIDDEN = 1024
QKV = NHEADS * HEAD_DIM  # 1024
SCALING = HEAD_DIM ** -0.5
B = 2
S = 2048
T = B * S  # 4096
NCORES = 8
HPC = NHEADS // NCORES  # 2 heads per core
FEAT = HPC * HEAD_DIM  # 128
CHUNK = 512
NCHUNK = S // CHUNK  # 4 per batch
KSLABS = HIDDEN // 128  # 8
SSLABS = S // 128  # 16
D1 = HEAD_DIM + 1  # 65

# toggles for A/B experiments
TILED_SCORES = True   # fused-head 64x128 row-tiled score pairs
MERGED_VTRANS = True  # [128,128] both-heads V transposes (32 vs 64)

_CACHE = {}
LAST_RESULT = None  # BassKernelResults of the most recent kernel() call


def _split_waits(nc, keep=1):
    """Hoist excess per-instruction sem waits into standalone EventSemaphore
    instructions (walrus codegen has small per-opcode wait budgets)."""
    import bass_rust
    import concourse.mybir as mybir

    n_new = 0
    for f in nc.m.functions:
        for blk in f.blocks:
            out = []
            changed = False
            for inst in blk.instructions:
                si = inst.sync_info
                waits = list(si.on_wait) if si is not None else []
                if len(waits) > keep:
                    excess = waits[: len(waits) - keep]
                    kept = waits[len(waits) - keep:]
                    for w in excess:
                        out.append(mybir.InstEventSemaphore(
                            name=f"{inst.name}-esw{n_new}",
                            engine=inst.engine,
                            sync_info=bass_rust.SyncInfo(on_wait=[w], on_update=[]),
                        ))
                        n_new += 1
                    inst.sync_info = bass_rust.SyncInfo(
                        on_wait=kept, on_update=list(si.on_update))
                    changed = True
                out.append(inst)
            if changed:
                blk.instructions = out
    return n_new


def _build(reps=1):
    import concourse.bass as bass
    import concourse.mybir as mybir
    import concourse.tile as tile

    from concourse.masks import make_identity
    f32 = mybir.dt.float32
    f32r = mybir.dt.float32r
    f16 = mybir.dt.float16
    Exp = mybir.ActivationFunctionType.Exp
    Ln = mybir.ActivationFunctionType.Ln

    nc = bass.Bass()
    xT = nc.dram_tensor("xT", [HIDDEN, T], f16, kind="ExternalInput")
    wqkvT = nc.dram_tensor("wqkvT", [HIDDEN, 3 * FEAT], f16, kind="ExternalInput")
    bqkv = nc.dram_tensor("bqkv", [FEAT, 3], f32, kind="ExternalInput")
    woT = nc.dram_tensor("woT", [FEAT, HIDDEN], f16, kind="ExternalInput")
    out_d = nc.dram_tensor("out", [T, HIDDEN], f16, kind="ExternalOutput")

    with tile.TileContext(nc) as tc, nc.allow_low_precision(reason="fp16 matmuls"):
        with (
            tc.tile_pool(name="sing", bufs=1) as sing,
            tc.tile_pool(name="xp", bufs=4) as xp,
            tc.tile_pool(name="pp", bufs=3) as pp,
            tc.tile_pool(name="stg", bufs=4) as stg,
            tc.tile_pool(name="sm", bufs=2) as sm,
            tc.tile_pool(name="dnp", bufs=2) as dnp,
            tc.tile_pool(name="op", bufs=2) as op,
            tc.tile_pool(name="ps_mm", bufs=2, space="PSUM") as ps_mm,
            tc.tile_pool(name="ps_s", bufs=2, space="PSUM") as ps_s,
            tc.tile_pool(name="ps_o", bufs=2, space="PSUM") as ps_o,
        ):
            wq_sb = sing.tile([128, KSLABS, 3 * FEAT], f16, tag="wq")
            wo_sb = sing.tile([FEAT, HIDDEN], f16, tag="wo")
            bq_sb = sing.tile([FEAT, 3], f32, tag="bq")
            QT = sing.tile([128, T], f16, tag="qt")
            KT = sing.tile([128, T], f16, tag="kt")
            VT = sing.tile([128, T], f32, tag="vt")
            OT = sing.tile([128, T], f16, tag="ot")
            Vaug = sing.tile([128, B, HPC, SSLABS, D1], f16, tag="va")

            def qkv_dma_early(n, split=1):
                """Chunk DMA, optionally split by k-slab halves so the first
                qkv matmuls can start before the whole chunk lands."""
                xc = xp.tile([128, KSLABS, CHUNK], f16, tag="xc", name="xc")
                xr = xT[:].rearrange("(s p) t -> p s t", p=128)
                sl = KSLABS // split
                for i in range(split):
                    nc.gpsimd.dma_start(
                        out=xc[:, i * sl:(i + 1) * sl, :],
                        in_=xr[:, i * sl:(i + 1) * sl,
                               n * CHUNK:(n + 1) * CHUNK])
                return xc

            wq_r = wqkvT[:].rearrange("(s p) f -> p s f", p=128)
            # finest splits first: MM(slab s) unblocks as soon as its own
            # wq+xc regions land (region-granular tile deps)
            nc.sync.dma_start(out=wq_sb[:, 0:2, 0:FEAT],
                              in_=wq_r[:, 0:2, 0:FEAT])
            xc01 = [qkv_dma_early(0, split=4), qkv_dma_early(1)]
            nc.sync.dma_start(out=wq_sb[:, 2:4, 0:FEAT],
                              in_=wq_r[:, 2:4, 0:FEAT])
            nc.sync.dma_start(out=wq_sb[:, 4:KSLABS, 0:FEAT],
                              in_=wq_r[:, 4:KSLABS, 0:FEAT])
            for m3 in range(1, 3):
                nc.sync.dma_start(out=wq_sb[:, :, m3 * FEAT:(m3 + 1) * FEAT],
                                  in_=wq_r[:, :, m3 * FEAT:(m3 + 1) * FEAT])
            nc.sync.dma_start(out=bq_sb, in_=bqkv[:])
            nc.sync.dma_start(out=wo_sb, in_=woT[:])
            ident = sing.tile([128, 128], f32, tag="id")
            make_identity(nc, ident)
            # e64a: ones at row 64 — e64a.T @ dn broadcasts the den row to
            # 64 output partitions; used col-tiled per head
            e64a = sing.tile([D1, HEAD_DIM], f16, tag="e64")
            dn2 = sing.tile([D1, 2, 2 * CHUNK], f16, tag="dn2")
            nc.vector.memset(e64a, 0.0)
            nc.vector.memset(e64a[HEAD_DIM:D1, :], 1.0)
            nc.vector.memset(dn2, 0.0)
            vst = stg.tile([128, B * HPC * SSLABS], f32, tag="vst")
            nc.vector.memset(vst, 1.0)
            nc.vector.tensor_copy(Vaug[:, :, :, :, HEAD_DIM:D1], vst)

            xT_c = xT[:].rearrange("(s p) t -> p s t", p=128)

            from collections import deque
            filler = deque()
            fillmm = deque()  # items guaranteed to start with a PE matmul

            def drain(n=1):
                for _ in range(n):
                    if filler:
                        filler.popleft()()
                    elif fillmm:
                        fillmm.popleft()()

            def drain_mm(n=1):
                for _ in range(n):
                    if fillmm:
                        fillmm.popleft()()
                    elif filler:
                        filler.popleft()()

            # emission-order guards: count-based drain pacing alone cannot
            # guarantee a consumer is EMITTED after its producer thunk (tile
            # deps only see already-emitted instructions)
            ready = set()

            def mark(key):
                ready.add(key)

            def need(key):
                while key not in ready:
                    assert filler or fillmm, f"need({key}): queues empty"
                    drain(1)

            def qkv_dma(b, n):
                g = b * NCHUNK + n
                xc = xp.tile([128, KSLABS, CHUNK], f16, tag="xc", name="xc")
                for i in range(2):
                    nc.gpsimd.dma_start(
                        out=xc[:, 4 * i:4 * i + 4, :],
                        in_=xT_c[:, 4 * i:4 * i + 4,
                                 g * CHUNK:(g + 1) * CHUNK])
                return xc

            def qkv_feat_half(b, n, m, xc, half, box):
                g = b * NCHUNK + n
                lo, hi = g * CHUNK, (g + 1) * CHUNK
                dest = (QT, KT, VT)[m]
                if half == 0:
                    box["acc"] = ps_mm.tile([128, CHUNK], f32, tag="mm",
                                            name="acc")
                acc = box["acc"]
                s0 = half * (KSLABS // 2)
                for s in range(s0, s0 + KSLABS // 2):
                    nc.tensor.matmul(
                        acc, wq_sb[:, s, m * FEAT:(m + 1) * FEAT], xc[:, s, :],
                        start=(s == 0), stop=(s == KSLABS - 1))
                if half == 1:
                    nc.vector.tensor_scalar_add(
                        dest[:, lo:hi], acc, bq_sb[:, m:m + 1])

            def qkv_feat(b, n, m, xc):
                box = {}
                qkv_feat_half(b, n, m, xc, 0, box)
                qkv_feat_half(b, n, m, xc, 1, box)

            def vtrans_k(b, k):
                """[128,128] PE transpose covering both heads at once."""
                tp = ps_mm.tile([128, CHUNK], f32, tag="mm", name="tp")
                nc.tensor.transpose(
                    tp[:, 0:128],
                    VT[:, b * S + 128 * k: b * S + 128 * (k + 1)],
                    ident)
                nc.vector.tensor_copy(
                    Vaug[:, b, :, k, 0:HEAD_DIM],
                    tp[:, 0:128].rearrange("p (h d) -> p h d", h=HPC))

            def norm_thunks(b, qc, o_ps, oproj_group=None):
                """Detached normalization, 4 phases dropped at kk=0/2/4/6 of
                the NEXT unit. Phase 0 (all DVE) stashes unnormalized OT and
                the two den rows to SBUF, releasing the o_ps PSUM banks
                ~2.6us into the next unit without touching ACT. Ln / rec /
                in-place fp16 muls then run with no PSUM coupling, so the
                softmax exps stream uninterrupted and the PE never idles
                long enough to re-throttle. The unit's oproj group is
                enqueued from inside the muls thunk (emission-order-safe)."""
                qlo = b * S + qc * CHUNK
                qsl = slice(qlo, qlo + CHUNK)
                par = qc % 2
                box = {}

                def stash():
                    for h in range(HPC):
                        nc.vector.tensor_copy(
                            OT[64 * h:64 * h + 64, qsl],
                            o_ps[h][0:HEAD_DIM, :])
                        nc.vector.tensor_copy(
                            dn2[HEAD_DIM:D1, par,
                                h * CHUNK:(h + 1) * CHUNK],
                            o_ps[h][HEAD_DIM:D1, :])

                def t_ln():
                    # broadcast-first: den rows fan out to all 128
                    # partitions via two col-tiled ones-row matmuls, then
                    # Ln and exp run as single full-width [128,512] ACT ops
                    b_ps = ps_mm.tile([128, CHUNK], f32, tag="mm",
                                      name="bps")
                    for h in range(HPC):
                        nc.tensor.matmul(
                            b_ps[64 * h:64 * h + 64, :], e64a,
                            dn2[:, par, h * CHUNK:(h + 1) * CHUNK],
                            start=True, stop=True)
                    lnt = sm.tile([128, CHUNK], f32, tag="ln", name="lnt")
                    nc.scalar.activation(out=lnt, in_=b_ps, func=Ln)
                    box["lnt"] = lnt

                def t_rec():
                    rb = sm.tile([128, CHUNK], f16, tag="rb", name="rb")
                    nc.scalar.activation(out=rb, in_=box["lnt"], func=Exp,
                                         scale=-1.0)
                    box["rb"] = rb

                def t_mul():
                    for h in range(HPC):
                        nc.vector.tensor_mul(
                            OT[64 * h:64 * h + 64, qsl],
                            OT[64 * h:64 * h + 64, qsl],
                            box["rb"][64 * h:64 * h + 64, :])
                    if oproj_group is not None:
                        fillmm.extend(oproj_group_thunks(oproj_group))

                def t_rec_mul():
                    t_rec()
                    t_mul()

                return [stash, t_ln, t_rec_mul]

            def attn_unit(b, qc, extra_drain=1, finish_prev=None,
                          last=False):
                """Fused-head unit: 16 k-slab steps, software-pipelined."""
                qlo = b * S + qc * CHUNK
                qsl = slice(qlo, qlo + CHUNK)
                o_ps = [ps_o.tile([D1, CHUNK], f32, tag="o",
                                  name=f"o{h}_{b}_{qc}") for h in range(HPC)]

                def s_step(k):
                    s_ps = ps_s.tile([128, HPC, CHUNK], f32, tag="s",
                                     name="s_ps")
                    for h in range(HPC):
                        nc.tensor.matmul(
                            s_ps[:, h, :],
                            KT[64 * h:64 * h + 64,
                               b * S + 128 * k: b * S + 128 * (k + 1)],
                            QT[64 * h:64 * h + 64, qsl],
                            start=True, stop=True)
                    ptk = pp.tile([128, HPC, CHUNK], f16, tag="pt", name="pt")
                    nc.scalar.activation(out=ptk, in_=s_ps, func=Exp)
                    return ptk

                def pv_step(k, ptk):
                    for h in range(HPC):
                        nc.tensor.matmul(
                            o_ps[h], Vaug[:, b, h, k, :], ptk[:, h, :],
                            start=(k == 0), stop=(k == SSLABS - 1))

                # 2-slab groups: [sc(k), sc(k+1)] then [pv(k), pv(k+1)],
                # pipelined one group deep — keeps TensorE in each tile mode
                # for 4 matmuls at a time (mode switches are ~100-250ns).
                need(("qt", b, qc))
                prev = None
                for kk in range(0, SSLABS, 2):
                    need(("kt", b, (kk + 1) // 4))
                    cur = (s_step(kk), s_step(kk + 1))
                    if finish_prev is not None and kk < 6:
                        finish_prev[kk // 2]()
                        if kk == 0:
                            drain_mm(2)
                    drain(extra_drain)
                    if prev is not None:
                        need(("va", b, (kk - 1) // 4))
                        pv_step(kk - 2, prev[0])
                        pv_step(kk - 1, prev[1])
                        drain(extra_drain)
                    prev = cur
                need(("va", b, NCHUNK - 1))
                pv_step(SSLABS - 2, prev[0])
                drain(extra_drain)
                if not last:
                    pv_step(SSLABS - 1, prev[1])
                    return norm_thunks(b, qc, o_ps,
                                       oproj_group=b * NCHUNK + qc)

                # Final unit: shortest-latency tail. Den copies interleaved
                # between the last PVs, broadcast-first norm, then the oproj
                # group inline with DVE/ACT-alternating casts.
                k15 = SSLABS - 1
                par = qc % 2
                nc.tensor.matmul(o_ps[0], Vaug[:, b, 0, k15, :],
                                 prev[1][:, 0, :], start=False, stop=True)
                nc.vector.tensor_copy(dn2[HEAD_DIM:D1, par, 0:CHUNK],
                                      o_ps[0][HEAD_DIM:D1, :])
                nc.tensor.matmul(o_ps[1], Vaug[:, b, 1, k15, :],
                                 prev[1][:, 1, :], start=False, stop=True)
                nc.vector.tensor_copy(dn2[HEAD_DIM:D1, par, CHUNK:2 * CHUNK],
                                      o_ps[1][HEAD_DIM:D1, :])
                b_ps = ps_mm.tile([128, CHUNK], f32, tag="mm", name="bps")
                for h in range(HPC):
                    nc.tensor.matmul(
                        b_ps[64 * h:64 * h + 64, :], e64a,
                        dn2[:, par, h * CHUNK:(h + 1) * CHUNK],
                        start=True, stop=True)
                lnt = sm.tile([128, CHUNK], f32, tag="ln", name="lntL")
                nc.scalar.activation(out=lnt, in_=b_ps, func=Ln)
                rb = sm.tile([128, CHUNK], f32, tag="rb", name="rb")
                nc.scalar.activation(out=rb, in_=lnt, func=Exp, scale=-1.0)
                for h in range(HPC):
                    nc.vector.tensor_mul(
                        OT[64 * h:64 * h + 64, qsl], o_ps[h][0:HEAD_DIM, :],
                        rb[64 * h:64 * h + 64, :])
                g = b * NCHUNK + qc
                ost = op.tile([128, 4, HIDDEN], f16, tag="ost", name="ostL")
                for jj in range(4):
                    for nh in range(HIDDEN // CHUNK):
                        oproj_half(g, jj, nh, ost,
                                   cast_scalar=(nh == 1))
                    nc.sync.dma_start(
                        out=out_d[512 * g + 128 * jj:
                                  512 * g + 128 * (jj + 1), :],
                        in_=ost[:, jj, :])
                return None

            def oproj_half(j, jj, nh, ost, cast_scalar=False):
                t = 4 * j + jj
                acc = ps_mm.tile([128, CHUNK], f32, tag="mm", name="acc2")
                nc.tensor.matmul(
                    acc, OT[:, 128 * t:128 * (t + 1)],
                    wo_sb[:, nh * CHUNK:(nh + 1) * CHUNK],
                    start=True, stop=True)
                if cast_scalar:
                    nc.scalar.copy(
                        ost[:, jj, nh * CHUNK:(nh + 1) * CHUNK], acc)
                else:
                    nc.vector.tensor_copy(
                        ost[:, jj, nh * CHUNK:(nh + 1) * CHUNK], acc)

            def oproj_group_thunks(j):
                # token tiles 4j..4j+3 (tokens 512j..512j+512); per-jj
                # flush DMAs so the tail cast/DMA pipeline stays overlapped
                box = {}

                def first():
                    box["ost"] = op.tile([128, 4, HIDDEN], f16, tag="ost",
                                         name="ost")
                    oproj_half(j, 0, 0, box["ost"])

                def flush1(jj):
                    nc.sync.dma_start(
                        out=out_d[512 * j + 128 * jj:512 * j + 128 * (jj + 1),
                                  :],
                        in_=box["ost"][:, jj, :])

                thunks = []
                for jj in range(4):
                    for nh in range(HIDDEN // CHUNK):
                        if jj == 0 and nh == 0:
                            thunks.append(first)
                        else:
                            thunks.append(
                                lambda jj=jj, nh=nh:
                                oproj_half(j, jj, nh, box["ost"]))
                    thunks.append(lambda jj=jj: flush1(jj))
                return thunks

            def queue_chunk(b, n, xc=None):
                """Queue one qkv chunk (dma + 3 projections + V transposes)."""
                box = {}

                if xc is not None:
                    box["xc"] = xc
                else:
                    def dma_thunk():
                        box["xc"] = qkv_dma(b, n)

                    filler.append(dma_thunk)
                for m in range(3):
                    fbox = {}

                    def feat_thunk(m=m, fbox=fbox, half=None):
                        qkv_feat_half(b, n, m, box["xc"], half, fbox)
                        if half == 1:
                            mark((("qt", "kt", "vt")[m], b, n))

                    for half in range(2):
                        filler.append(
                            lambda half=half, ft=feat_thunk: ft(half=half))

                def vt_thunk(k):
                    vtrans_k(b, k)
                    if k == 4 * n + 3:
                        mark(("va", b, n))

                for k in range(4 * n, 4 * n + 4):
                    filler.append(lambda k=k: vt_thunk(k))

            # ---- emission: qkv(b0 ch0-1) up front, then fused-head units
            # with fine-grained filler (qkv / vtrans / norm / oproj) drained
            # inside exp shadows ----
            for _rep in range(reps):
                # inline only chunk-0 q/k feats — the minimum for unit 0's
                # first score pair; v-feats + vtranses drain as the first
                # fillers inside unit 0 (readiness guards enforce emission
                # order), so the exp pipeline starts ~10us earlier
                xc0 = xc01[0] if _rep == 0 else qkv_dma(0, 0)
                for m in range(2):
                    qkv_feat(0, 0, m, xc0)
                    mark((("qt", "kt")[m], 0, 0))
                vbox = {}

                def v0_thunk(half):
                    qkv_feat_half(0, 0, 2, xc0, half, vbox)
                    if half == 1:
                        mark(("vt", 0, 0))

                for half in range(2):
                    filler.append(lambda half=half: v0_thunk(half))

                def vt0_thunk(k):
                    vtrans_k(0, k)
                    if k == 3:
                        mark(("va", 0, 0))

                for k in range(4):
                    filler.append(lambda k=k: vt0_thunk(k))

                queue_chunk(0, 1, xc=xc01[1] if _rep == 0 else None)
                for n in range(2, NCHUNK):
                    queue_chunk(0, n)
                for n in range(NCHUNK):
                    queue_chunk(1, n)

                fin = None
                for qc in range(NCHUNK):  # batch 0
                    fin = attn_unit(0, qc,
                                    extra_drain=2 if qc < 2 else 1,
                                    finish_prev=fin)
                for qc in range(NCHUNK):  # batch 1
                    fin = attn_unit(1, qc, finish_prev=fin,
                                    extra_drain=2 if qc == NCHUNK - 1 else 1,
                                    last=(qc == NCHUNK - 1))
                assert fin is None
                while filler or fillmm:
                    drain(1)

    _split_waits(nc)
    return nc


def _make_in_maps(hidden_states, w_qkv, b_qkv, w_o, b_o):
    x16 = np.ascontiguousarray(
        np.asarray(hidden_states, dtype=np.float32).reshape(T, HIDDEN).T
    ).astype(np.float16)
    w_qkv = np.asarray(w_qkv, dtype=np.float32)
    b_qkv = np.asarray(b_qkv, dtype=np.float32)
    w_o = np.asarray(w_o, dtype=np.float32)

    in_maps = []
    for c in range(NCORES):
        rq = slice(c * FEAT, (c + 1) * FEAT)
        wq = w_qkv[0:QKV][rq] * SCALING
        wk = w_qkv[QKV:2 * QKV][rq]
        wv = w_qkv[2 * QKV:3 * QKV][rq]
        bq = b_qkv[0:QKV][rq] * SCALING
        bk = b_qkv[QKV:2 * QKV][rq]
        bv = b_qkv[2 * QKV:3 * QKV][rq]
        in_maps.append({
            "xT": x16,
            "wqkvT": np.ascontiguousarray(
                np.concatenate([wq, wk, wv], axis=0).T).astype(np.float16),
            "bqkv": np.ascontiguousarray(np.stack([bq, bk, bv], axis=1)),
            "woT": np.ascontiguousarray(w_o[:, rq].T).astype(np.float16),
        })
    return in_maps


def kernel(hidden_states, w_qkv, b_qkv, w_o, b_o):
    global LAST_RESULT
    from concourse.bass_utils import run_bass_kernel_spmd
    import os

    if "nc" not in _CACHE:
        _CACHE["nc"] = _build()
    nc = _CACHE["nc"]

    b_o = np.asarray(b_o, dtype=np.float32)
    in_maps = _make_in_maps(hidden_states, w_qkv, b_qkv, w_o, b_o)

    trace = bool(os.environ.get("KERNEL_TRACE"))
    res = run_bass_kernel_spmd(nc, in_maps, list(range(NCORES)), trace=trace)
    LAST_RESULT = res

    acc = np.zeros((T, HIDDEN), dtype=np.float32)
    for c in range(NCORES):
        acc += res.results[c]["out"]
    out = (acc + b_o).astype(np.float32).reshape(B, S, HIDDEN)
    return out



# revision 66
# speedup vs baseline: 1.0195x; 1.0030x over previous
"""Multi-head attention (B=2, S=2048, H=1024, 16 heads x 64d) on 8 trn2 cores.

Sharding: tensor-parallel over heads (2 heads/core). Each core computes the
qkv projection for its 384 output features, attention for its 2 heads, and a
partial o_proj ([4096,1024] over its 128-feature slice). Host sums the 8
partials (fp16) and adds b_o.

v2 (fused heads): the two heads are processed in lockstep per (batch, query
chunk) unit. Scores for h0/h1 are emitted back-to-back as 64x128 row-tiled
matmuls at tile positions (0,0)/(64,0) into separate PSUM banks so the PE
array halves can overlap; one [128,1024] exp covers both heads per k-slab.
o_proj output and final host sum are fp16. Emission software-pipelines S/PV
around the exp and drains a fine-grained filler queue (qkv chunks /
V transposes / o_proj) inside exp shadows.

v3 (detached normalization, 306us -> 234us): the per-unit softmax
normalization is decoupled from the PSUM accumulators and from the exp
stream. At the next unit's kk=0 a DVE-only stash copies unnormalized OT and
the two den rows to SBUF (releasing the o_ps banks ~2.6us in, in parallel
with the exps); Ln / rec-exp+broadcast / in-place fp16 muls then run as
thunks at kk=2/4 with no PSUM coupling. The baseline emitted 2xLn + rec-exp
between the last PV and the next unit's scores, which stalled the PE ~2.1us
per unit and HAM-rethrottled the clock to 1.2GHz for ~3-7us afterwards
(64us of throttled PE time -> ~19us). Other changes: [128,128] merged
both-heads V transposes (32 instead of 64), fp16 e01/rec2 broadcast matmul,
region-split first DMAs so the first qkv matmul starts ~4us earlier, only
chunk 0 inline before unit 0 (first exp at ~25us instead of ~39us), each
unit's oproj group enqueued from its muls thunk (emission-order safe),
per-token-tile flush DMAs, and a short-latency inline tail for the final
unit (PSUM-direct norm + DVE/ACT-alternating oproj casts).

v4 (~226us): broadcast-FIRST normalization — the two fp16 den rows are
fanned out to all 128 partitions by col-tiled ones-row matmuls (e64a)
BEFORE the Ln/exp, so both ACT ops run as single full-width [128,512]
instructions (1.18us/unit vs 1.95us) and the separate rb copy disappears;
mark()/need() readiness guards enforce producer-thunk emission order
(count-based drain pacing alone is unsound).
"""
import sys

sys.path.insert(0, "/opt/trn_rl_repo")
import numpy as np

NHEADS = 16
HEAD_DIM = 64
HIDDEN = 1024
QKV = NHEADS * HEAD_DIM  # 1024
SCALING = HEAD_DIM ** -0.5
B = 2
S = 2048
T = B * S  # 4096
NCORES = 8
HPC = NHEADS // NCORES  # 2 heads per core
FEAT = HPC * HEAD_DIM  # 128
CHUNK = 512
NCHUNK = S // CHUNK  # 4 per batch
KSLABS = HIDDEN // 128  # 8
SSLABS = S // 128  # 16
D1 = HEAD_DIM + 1  # 65

# toggles for A/B experiments
TILED_SCORES = True   # fused-head 64x128 row-tiled score pairs
MERGED_VTRANS = True  # [128,128] both-heads V transposes (32 vs 64)

_CACHE = {}
LAST_RESULT = None  # BassKernelResults of the most recent kernel() call


def _split_waits(nc, keep=1):
    """Hoist excess per-instruction sem waits into standalone EventSemaphore
    instructions (walrus codegen has small per-opcode wait budgets)."""
    import bass_rust
    import concourse.mybir as mybir

    n_new = 0
    for f in nc.m.functions:
        for blk in f.blocks:
            out = []
            changed = False
            for inst in blk.instructions:
                si = inst.sync_info
                waits = list(si.on_wait) if si is not None else []
                if len(waits) > keep:
                    excess = waits[: len(waits) - keep]
                    kept = waits[len(waits) - keep:]
                    for w in excess:
                        out.append(mybir.InstEventSemaphore(
                            name=f"{inst.name}-esw{n_new}",
                            engine=inst.engine,
                            sync_info=bass_rust.SyncInfo(on_wait=[w], on_update=[]),
                        ))
                        n_new += 1
                    inst.sync_info = bass_rust.SyncInfo(
                        on_wait=kept, on_update=list(si.on_update))
                    changed = True
                out.append(inst)
            if changed:
                blk.instructions = out
    return n_new


def _build(reps=1):
    import concourse.bass as bass
    import concourse.mybir as mybir
    import concourse.tile as tile

    from concourse.masks import make_identity
    f32 = mybir.dt.float32
    f32r = mybir.dt.float32r
    f16 = mybir.dt.float16
    Exp = mybir.ActivationFunctionType.Exp
    Ln = mybir.ActivationFunctionType.Ln

    nc = bass.Bass()
    xT = nc.dram_tensor("xT", [HIDDEN, T], f16, kind="ExternalInput")
    wqkvT = nc.dram_tensor("wqkvT", [HIDDEN, 3 * FEAT], f16, kind="ExternalInput")
    bqkv = nc.dram_tensor("bqkv", [FEAT, 3], f32, kind="ExternalInput")
    woT = nc.dram_tensor("woT", [FEAT, HIDDEN], f16, kind="ExternalInput")
    out_d = nc.dram_tensor("out", [T, HIDDEN], f16, kind="ExternalOutput")

    with tile.TileContext(nc) as tc, nc.allow_low_precision(reason="fp16 matmuls"):
        with (
            tc.tile_pool(name="sing", bufs=1) as sing,
            tc.tile_pool(name="xp", bufs=4) as xp,
            tc.tile_pool(name="pp", bufs=3) as pp,
            tc.tile_pool(name="stg", bufs=4) as stg,
            tc.tile_pool(name="sm", bufs=2) as sm,
            tc.tile_pool(name="dnp", bufs=2) as dnp,
            tc.tile_pool(name="op", bufs=2) as op,
            tc.tile_pool(name="ps_mm", bufs=2, space="PSUM") as ps_mm,
            tc.tile_pool(name="ps_s", bufs=2, space="PSUM") as ps_s,
            tc.tile_pool(name="ps_o", bufs=2, space="PSUM") as ps_o,
        ):
            wq_sb = sing.tile([128, KSLABS, 3 * FEAT], f16, tag="wq")
            wo_sb = sing.tile([FEAT, HIDDEN], f16, tag="wo")
            bq_sb = sing.tile([FEAT, 3], f32, tag="bq")
            QT = sing.tile([128, T], f16, tag="qt")
            KT = sing.tile([128, T], f16, tag="kt")
            VT = sing.tile([128, T], f32, tag="vt")
            OT = sing.tile([128, T], f16, tag="ot")
            Vaug = sing.tile([128, B, HPC, SSLABS, D1], f16, tag="va")

            def qkv_dma_early(n, split=1):
                """Chunk DMA, optionally split by k-slab halves so the first
                qkv matmuls can start before the whole chunk lands."""
                xc = xp.tile([128, KSLABS, CHUNK], f16, tag="xc", name="xc")
                xr = xT[:].rearrange("(s p) t -> p s t", p=128)
                sl = KSLABS // split
                for i in range(split):
                    nc.gpsimd.dma_start(
                        out=xc[:, i * sl:(i + 1) * sl, :],
                        in_=xr[:, i * sl:(i + 1) * sl,
                               n * CHUNK:(n + 1) * CHUNK])
                return xc

            wq_r = wqkvT[:].rearrange("(s p) f -> p s f", p=128)
            # finest splits first: MM(slab s) unblocks as soon as its own
            # wq+xc regions land (region-granular tile deps)
            nc.sync.dma_start(out=wq_sb[:, 0:2, 0:FEAT],
                              in_=wq_r[:, 0:2, 0:FEAT])
            xc01 = [qkv_dma_early(0, split=4), qkv_dma_early(1)]
            nc.sync.dma_start(out=wq_sb[:, 2:4, 0:FEAT],
                              in_=wq_r[:, 2:4, 0:FEAT])
            nc.sync.dma_start(out=wq_sb[:, 4:KSLABS, 0:FEAT],
                              in_=wq_r[:, 4:KSLABS, 0:FEAT])
            for m3 in range(1, 3):
                nc.sync.dma_start(out=wq_sb[:, :, m3 * FEAT:(m3 + 1) * FEAT],
                                  in_=wq_r[:, :, m3 * FEAT:(m3 + 1) * FEAT])
            nc.sync.dma_start(out=bq_sb, in_=bqkv[:])
            nc.sync.dma_start(out=wo_sb, in_=woT[:])
            ident = sing.tile([128, 128], f32, tag="id")
            make_identity(nc, ident)
            # e64a: ones at row 64 — e64a.T @ dn broadcasts the den row to
            # 64 output partitions; used col-tiled per head
            e64a = sing.tile([D1, HEAD_DIM], f16, tag="e64")
            dn2 = sing.tile([D1, 2, 2 * CHUNK], f16, tag="dn2")
            nc.vector.memset(e64a, 0.0)
            nc.vector.memset(e64a[HEAD_DIM:D1, :], 1.0)
            nc.vector.memset(dn2, 0.0)
            vst = stg.tile([128, B * HPC * SSLABS], f32, tag="vst")
            nc.vector.memset(vst, 1.0)
            nc.vector.tensor_copy(Vaug[:, :, :, :, HEAD_DIM:D1], vst)

            xT_c = xT[:].rearrange("(s p) t -> p s t", p=128)

            from collections import deque
            filler = deque()
            fillmm = deque()  # items guaranteed to start with a PE matmul

            def drain(n=1):
                for _ in range(n):
                    if filler:
                        filler.popleft()()
                    elif fillmm:
                        fillmm.popleft()()

            def drain_mm(n=1):
                for _ in range(n):
                    if fillmm:
                        fillmm.popleft()()
                    elif filler:
                        filler.popleft()()

            # emission-order guards: count-based drain pacing alone cannot
            # guarantee a consumer is EMITTED after its producer thunk (tile
            # deps only see already-emitted instructions)
            ready = set()

            def mark(key):
                ready.add(key)

            def need(key):
                while key not in ready:
                    assert filler or fillmm, f"need({key}): queues empty"
                    drain(1)

            def qkv_dma(b, n):
                g = b * NCHUNK + n
                xc = xp.tile([128, KSLABS, CHUNK], f16, tag="xc", name="xc")
                for i in range(2):
                    nc.gpsimd.dma_start(
                        out=xc[:, 4 * i:4 * i + 4, :],
                        in_=xT_c[:, 4 * i:4 * i + 4,
                                 g * CHUNK:(g + 1) * CHUNK])
                return xc

            def qkv_feat_half(b, n, m, xc, half, box):
                g = b * NCHUNK + n
                lo, hi = g * CHUNK, (g + 1) * CHUNK
                dest = (QT, KT, VT)[m]
                if half == 0:
                    box["acc"] = ps_mm.tile([128, CHUNK], f32, tag="mm",
                                            name="acc")
                acc = box["acc"]
                s0 = half * (KSLABS // 2)
                for s in range(s0, s0 + KSLABS // 2):
                    nc.tensor.matmul(
                        acc, wq_sb[:, s, m * FEAT:(m + 1) * FEAT], xc[:, s, :],
                        start=(s == 0), stop=(s == KSLABS - 1))
                if half == 1:
                    nc.vector.tensor_scalar_add(
                        dest[:, lo:hi], acc, bq_sb[:, m:m + 1])

            def qkv_feat(b, n, m, xc):
                box = {}
                qkv_feat_half(b, n, m, xc, 0, box)
                qkv_feat_half(b, n, m, xc, 1, box)

            def vtrans_k(b, k):
                """[128,128] PE transpose covering both heads at once."""
                tp = ps_mm.tile([128, CHUNK], f32, tag="mm", name="tp")
                nc.tensor.transpose(
                    tp[:, 0:128],
                    VT[:, b * S + 128 * k: b * S + 128 * (k + 1)],
                    ident)
                nc.vector.tensor_copy(
                    Vaug[:, b, :, k, 0:HEAD_DIM],
                    tp[:, 0:128].rearrange("p (h d) -> p h d", h=HPC))

            def norm_thunks(b, qc, o_ps, oproj_group=None):
                """Detached normalization, 4 phases dropped at kk=0/2/4/6 of
                the NEXT unit. Phase 0 (all DVE) stashes unnormalized OT and
                the two den rows to SBUF, releasing the o_ps PSUM banks
                ~2.6us into the next unit without touching ACT. Ln / rec /
                in-place fp16 muls then run with no PSUM coupling, so the
                softmax exps stream uninterrupted and the PE never idles
                long enough to re-throttle. The unit's oproj group is
                enqueued from inside the muls thunk (emission-order-safe)."""
                qlo = b * S + qc * CHUNK
                qsl = slice(qlo, qlo + CHUNK)
                par = qc % 2
                box = {}

                def stash():
                    for h in range(HPC):
                        nc.vector.tensor_copy(
                            OT[64 * h:64 * h + 64, qsl],
                            o_ps[h][0:HEAD_DIM, :])
                        nc.vector.tensor_copy(
                            dn2[HEAD_DIM:D1, par,
                                h * CHUNK:(h + 1) * CHUNK],
                            o_ps[h][HEAD_DIM:D1, :])

                def t_ln():
                    # broadcast-first: den rows fan out to all 128
                    # partitions via two col-tiled ones-row matmuls, then
                    # Ln and exp run as single full-width [128,512] ACT ops
                    b_ps = ps_mm.tile([128, CHUNK], f32, tag="mm",
                                      name="bps")
                    for h in range(HPC):
                        nc.tensor.matmul(
                            b_ps[64 * h:64 * h + 64, :], e64a,
                            dn2[:, par, h * CHUNK:(h + 1) * CHUNK],
                            start=True, stop=True)
                    lnt = sm.tile([128, CHUNK], f32, tag="ln", name="lnt")
                    nc.scalar.activation(out=lnt, in_=b_ps, func=Ln)
                    box["lnt"] = lnt

                def t_rec():
                    rb = sm.tile([128, CHUNK], f16, tag="rb", name="rb")
                    nc.scalar.activation(out=rb, in_=box["lnt"], func=Exp,
                                         scale=-1.0)
                    box["rb"] = rb

                def t_mul():
                    for h in range(HPC):
                        nc.vector.tensor_mul(
                            OT[64 * h:64 * h + 64, qsl],
                            OT[64 * h:64 * h + 64, qsl],
                            box["rb"][64 * h:64 * h + 64, :])
                    if oproj_group is not None:
                        fillmm.extend(oproj_group_thunks(oproj_group))

                def t_rec_mul():
                    t_rec()
                    t_mul()

                return [stash, t_ln, t_rec_mul]

            def attn_unit(b, qc, extra_drain=1, finish_prev=None,
                          last=False):
                """Fused-head unit: 16 k-slab steps, software-pipelined."""
                qlo = b * S + qc * CHUNK
                qsl = slice(qlo, qlo + CHUNK)
                o_ps = [ps_o.tile([D1, CHUNK], f32, tag="o",
                                  name=f"o{h}_{b}_{qc}") for h in range(HPC)]

                def s_step(k):
                    s_ps = ps_s.tile([128, HPC, CHUNK], f32, tag="s",
                                     name="s_ps")
                    for h in range(HPC):
                        nc.tensor.matmul(
                            s_ps[:, h, :],
                            KT[64 * h:64 * h + 64,
                               b * S + 128 * k: b * S + 128 * (k + 1)],
                            QT[64 * h:64 * h + 64, qsl],
                            start=True, stop=True)
                    ptk = pp.tile([128, HPC, CHUNK], f16, tag="pt", name="pt")
                    nc.scalar.activation(out=ptk, in_=s_ps, func=Exp)
                    return ptk

                def pv_step(k, ptk):
                    for h in range(HPC):
                        nc.tensor.matmul(
                            o_ps[h], Vaug[:, b, h, k, :], ptk[:, h, :],
                            start=(k == 0), stop=(k == SSLABS - 1))

                # 2-slab groups: [sc(k), sc(k+1)] then [pv(k), pv(k+1)],
                # pipelined one group deep — keeps TensorE in each tile mode
                # for 4 matmuls at a time (mode switches are ~100-250ns).
                need(("qt", b, qc))
                prev = None
                for kk in range(0, SSLABS, 2):
                    need(("kt", b, (kk + 1) // 4))
                    cur = (s_step(kk), s_step(kk + 1))
                    if finish_prev is not None and kk < 6:
                        finish_prev[kk // 2]()
                        if kk == 0:
                            drain_mm(2)
                    drain(extra_drain)
                    if prev is not None:
                        need(("va", b, (kk - 1) // 4))
                        pv_step(kk - 2, prev[0])
                        pv_step(kk - 1, prev[1])
                        drain(extra_drain)
                    prev = cur
                need(("va", b, NCHUNK - 1))
                pv_step(SSLABS - 2, prev[0])
                drain(extra_drain)
                if not last:
                    pv_step(SSLABS - 1, prev[1])
                    return norm_thunks(b, qc, o_ps,
                                       oproj_group=b * NCHUNK + qc)

                # Final unit: shortest-latency tail. Den copies interleaved
                # between the last PVs, broadcast-first norm, then the oproj
                # group inline with DVE/ACT-alternating casts.
                k15 = SSLABS - 1
                par = qc % 2
                nc.tensor.matmul(o_ps[0], Vaug[:, b, 0, k15, :],
                                 prev[1][:, 0, :], start=False, stop=True)
                nc.vector.tensor_copy(dn2[HEAD_DIM:D1, par, 0:CHUNK],
                                      o_ps[0][HEAD_DIM:D1, :])
                nc.tensor.matmul(o_ps[1], Vaug[:, b, 1, k15, :],
                                 prev[1][:, 1, :], start=False, stop=True)
                nc.vector.tensor_copy(dn2[HEAD_DIM:D1, par, CHUNK:2 * CHUNK],
                                      o_ps[1][HEAD_DIM:D1, :])
                b_ps = ps_mm.tile([128, CHUNK], f32, tag="mm", name="bps")
                for h in range(HPC):
                    nc.tensor.matmul(
                        b_ps[64 * h:64 * h + 64, :], e64a,
                        dn2[:, par, h * CHUNK:(h + 1) * CHUNK],
                        start=True, stop=True)
                lnt = sm.tile([128, CHUNK], f32, tag="ln", name="lntL")
                nc.scalar.activation(out=lnt, in_=b_ps, func=Ln)
                rb = sm.tile([128, CHUNK], f32, tag="rb", name="rb")
                nc.scalar.activation(out=rb, in_=lnt, func=Exp, scale=-1.0)
                for h in range(HPC):
                    nc.vector.tensor_mul(
                        OT[64 * h:64 * h + 64, qsl], o_ps[h][0:HEAD_DIM, :],
                        rb[64 * h:64 * h + 64, :])
                g = b * NCHUNK + qc
                ost = op.tile([128, 4, HIDDEN], f16, tag="ost", name="ostL")
                for jj in range(4):
                    for nh in range(HIDDEN // CHUNK):
                        oproj_half(g, jj, nh, ost,
                                   cast_scalar=(nh == 1))
                    nc.sync.dma_start(
                        out=out_d[512 * g + 128 * jj:
                                  512 * g + 128 * (jj + 1), :],
                        in_=ost[:, jj, :])
                return None

            def oproj_half(j, jj, nh, ost, cast_scalar=False):
                t = 4 * j + jj
                acc = ps_mm.tile([128, CHUNK], f32, tag="mm", name="acc2")
                nc.tensor.matmul(
                    acc, OT[:, 128 * t:128 * (t + 1)],
                    wo_sb[:, nh * CHUNK:(nh + 1) * CHUNK],
                    start=True, stop=True)
                if cast_scalar:
                    nc.scalar.copy(
                        ost[:, jj, nh * CHUNK:(nh + 1) * CHUNK], acc)
                else:
                    nc.vector.tensor_copy(
                        ost[:, jj, nh * CHUNK:(nh + 1) * CHUNK], acc)

            def oproj_group_thunks(j):
                # token tiles 4j..4j+3 (tokens 512j..512j+512); per-jj
                # flush DMAs so the tail cast/DMA pipeline stays overlapped
                box = {}

                def first():
                    box["ost"] = op.tile([128, 4, HIDDEN], f16, tag="ost",
                                         name="ost")
                    oproj_half(j, 0, 0, box["ost"])

                def flush1(jj):
                    nc.sync.dma_start(
                        out=out_d[512 * j + 128 * jj:512 * j + 128 * (jj + 1),
                                  :],
                        in_=box["ost"][:, jj, :])

                thunks = []
                for jj in range(4):
                    for nh in range(HIDDEN // CHUNK):
                        if jj == 0 and nh == 0:
                            thunks.append(first)
                        else:
                            thunks.append(
                                lambda jj=jj, nh=nh:
                                oproj_half(j, jj, nh, box["ost"]))
                    thunks.append(lambda jj=jj: flush1(jj))
                return thunks

            def queue_chunk(b, n, xc=None):
                """Queue one qkv chunk (dma + 3 projections + V transposes)."""
                box = {}

                if xc is not None:
                    box["xc"] = xc
                else:
                    def dma_thunk():
                        box["xc"] = qkv_dma(b, n)

                    filler.append(dma_thunk)
                for m in range(3):
                    fbox = {}

                    def feat_thunk(m=m, fbox=fbox, half=None):
                        qkv_feat_half(b, n, m, box["xc"], half, fbox)
                        if half == 1:
                            mark((("qt", "kt", "vt")[m], b, n))

                    for half in range(2):
                        filler.append(
                            lambda half=half, ft=feat_thunk: ft(half=half))

                def vt_thunk(k):
                    vtrans_k(b, k)
                    if k == 4 * n + 3:
                        mark(("va", b, n))

                for k in range(4 * n, 4 * n + 4):
                    filler.append(lambda k=k: vt_thunk(k))

            # ---- emission: qkv(b0 ch0-1) up front, then fused-head units
            # with fine-grained filler (qkv / vtrans / norm / oproj) drained
            # inside exp shadows ----
            for _rep in range(reps):
                # inline only chunk-0 q/k feats — the minimum for unit 0's
                # first score pair; v-feats + vtranses drain as the first
                # fillers inside unit 0 (readiness guards enforce emission
                # order), so the exp pipeline starts ~10us earlier
                xc0 = xc01[0] if _rep == 0 else qkv_dma(0, 0)
                for m in range(2):
                    qkv_feat(0, 0, m, xc0)
                    mark((("qt", "kt")[m], 0, 0))
                vbox = {}

                def v0_thunk(half):
                    qkv_feat_half(0, 0, 2, xc0, half, vbox)
                    if half == 1:
                        mark(("vt", 0, 0))

                for half in range(2):
                    filler.append(lambda half=half: v0_thunk(half))

                def vt0_thunk(k):
                    vtrans_k(0, k)
                    if k == 3:
                        mark(("va", 0, 0))

                for k in range(4):
                    filler.append(lambda k=k: vt0_thunk(k))

                queue_chunk(0, 1, xc=xc01[1] if _rep == 0 else None)
                for n in range(2, NCHUNK):
                    queue_chunk(0, n)
                for n in range(NCHUNK):
                    queue_chunk(1, n)

                fin = None
                for qc in range(NCHUNK):  # batch 0
                    fin = attn_unit(0, qc,
                                    extra_drain=2 if qc < 2 else 1,
                                    finish_prev=fin)
                for qc in range(NCHUNK):  # batch 1
                    fin = attn_unit(1, qc, finish_prev=fin,
                                    extra_drain=2 if qc == NCHUNK - 1 else 1,
                                    last=(qc == NCHUNK - 1))
                assert fin is None
                while filler or fillmm:
                    drain(1)

    _split_waits(nc)
    return nc


def _make_in_maps(hidden_states, w_qkv, b_qkv, w_o, b_o):
    x16 = np.ascontiguousarray(
        np.asarray(hidden_states, dtype=np.float32).reshape(T, HIDDEN).T
    ).astype(np.float16)
    w_qkv = np.asarray(w_qkv, dtype=np.float32)
    b_qkv = np.asarray(b_qkv, dtype=np.float32)
    w_o = np.asarray(w_o, dtype=np.float32)

    in_maps = []
    for c in range(NCORES):
        rq = slice(c * FEAT, (c + 1) * FEAT)
        wq = w_qkv[0:QKV][rq] * SCALING
        wk = w_qkv[QKV:2 * QKV][rq]
        wv = w_qkv[2 * QKV:3 * QKV][rq]
        bq = b_qkv[0:QKV][rq] * SCALING
        bk = b_qkv[QKV:2 * QKV][rq]
        bv = b_qkv[2 * QKV:3 * QKV][rq]
        in_maps.append({
            "xT": x16,
            "wqkvT": np.ascontiguousarray(
                np.concatenate([wq, wk, wv], axis=0).T).astype(np.float16),
            "bqkv": np.ascontiguousarray(np.stack([bq, bk, bv], axis=1)),
            "woT": np.ascontiguousarray(w_o[:, rq].T).astype(np.float16),
        })
    return in_maps


def kernel(hidden_states, w_qkv, b_qkv, w_o, b_o):
    global LAST_RESULT
    from concourse.bass_utils import run_bass_kernel_spmd
    import os

    if "nc" not in _CACHE:
        _CACHE["nc"] = _build()
    nc = _CACHE["nc"]

    b_o = np.asarray(b_o, dtype=np.float32)
    in_maps = _make_in_maps(hidden_states, w_qkv, b_qkv, w_o, b_o)

    trace = bool(os.environ.get("KERNEL_TRACE"))
    res = run_bass_kernel_spmd(nc, in_maps, list(range(NCORES)), trace=trace)
    LAST_RESULT = res

    acc = np.zeros((T, HIDDEN), dtype=np.float32)
    for c in range(NCORES):
        acc += res.results[c]["out"]
    out = (acc + b_o).astype(np.float32).reshape(B, S, HIDDEN)
    return out

